# revision 12
# baseline (speedup 1.0000x reference)
"""Trainium2 Bass kernel for nn_CrossLayer (DCN-style cross stack).

Reference semantics (B=16384, D=1024, L=8):
    out_0 = x
    s_i = einsum('bd,d->b', out_i, W[i])
    out_{i+1} = x * s_i[:, None] + b[i] + x

Algebraic collapse: out = x * rho_8 + b[L-1] with
    rho_1 = u_0 + 1,   rho_{l+1} = rho_l * u_l + c_l
    u_l[r] = <x[r, :], W[l]>          (U = x @ W.T, [B, L])
    c_l = <b[l-1], W[l]> + 1          (weights-only scalars)

Device pipeline (per 256-row tile, rows pair-packed 2/partition):
    1. DMA x tile [128, 2048] f32 (8KB contiguous per partition)
    2. ACT pack -> bf16 interleaved (d,r): u32 word = (row2p, row2p+1) pair
    3. PE transposes the u32-packed tile (8x 128x128) -> bf16 x^T, 2 rows/op
    4. PE matmul U^T = sum_c wt_c.T @ xT_c (bf16), small transposes back
    5. DVE scan -> rho_8 per row; y = x * rho + b7 emitted as bf16
    6. DMA store y bf16 (host upcasts to f32; rel err ~2^-9 << 2e-2 gate)

bf16 y stores halve write traffic: 8MB in + 4MB out per core ~= 31us
DMA floor at ~400GB/s. u32 pair-packing halves PE transpose count.

Sharding: data-parallel over batch; 8 cores x 2048 rows.
"""

import numpy as np

import concourse.bacc as bacc
import concourse.tile as tile
from concourse import mybir
from concourse.bass_utils import run_bass_kernel_spmd
from concourse.masks import make_identity

N_CORES = 8
B, D, L = 16384, 1024, 8
RPC = B // N_CORES          # rows per core (2048)
TPR = 2                     # rows packed per partition
NT = RPC // (128 * TPR)     # 256-row tiles per core (8)
NCH = D // 128              # 128-wide d chunks (8)
N_WARM = 16                 # bf16 warmup matmuls to lift HAM to K=8/8

LAST_RESULTS = None


def _build(cvals):
    """Trace + compile the per-core program. cvals = [c_1..c_{L-1}]."""
    nc = bacc.Bacc("TRN2", target_bir_lowering=False, debug=False)
    f32 = mybir.dt.float32
    bf16 = mybir.dt.bfloat16
    u32 = mybir.dt.uint32

    f16 = mybir.dt.float16

    x_d = nc.dram_tensor("x", [RPC, D], f32, kind="ExternalInput")
    # wt holds [Wh | Wl*2^11] per chunk: fp16 hi/lo split of W.T
    wt_d = nc.dram_tensor("wt", [128, NCH * 2 * L], f16, kind="ExternalInput")
    b7_d = nc.dram_tensor("b7r", [128, D], bf16, kind="ExternalInput")
    y_d = nc.dram_tensor("y", [RPC, D], bf16, kind="ExternalOutput")

    # row-pair tile views: [t][p, (r d)] -- 8KB contiguous per partition
    x_tile = x_d.ap().rearrange("(t p r) d -> t p (r d)", p=128, r=TPR)
    y_tile = y_d.ap().rearrange("(t p r) d -> t p (r d)", p=128, r=TPR)

    with tile.TileContext(nc) as tc:
        with (
            tc.tile_pool(name="const", bufs=1) as cpool,
            tc.tile_pool(name="xf", bufs=4) as xfpool,
            tc.tile_pool(name="pk", bufs=2) as pkpool,
            tc.tile_pool(name="xt", bufs=2) as xtpool,
            tc.tile_pool(name="yt", bufs=3) as ytpool,
            tc.tile_pool(name="small", bufs=6) as spool,
            tc.tile_pool(name="pst", bufs=2, space="PSUM") as pst,
            tc.tile_pool(name="psu", bufs=2, space="PSUM") as psu,
            tc.tile_pool(name="psr", bufs=2, space="PSUM") as psr,
        ):
            # --- first x data on the wire before anything else ---
            xf0 = xfpool.tile([128, TPR * D], f32, tag="xf")
            nc.sync.dma_start(out=xf0[:], in_=x_tile[0])
            xf1 = xfpool.tile([128, TPR * D], f32, tag="xf")
            nc.sync.dma_start(out=xf1[:], in_=x_tile[1])
            xf2 = xfpool.tile([128, TPR * D], f32, tag="xf")
            nc.sync.dma_start(out=xf2[:], in_=x_tile[2])

            # --- warmup: dense bf16 matmuls during initial DMA window ---
            dummy = cpool.tile([128, 512], bf16)
            nc.gpsimd.memset(dummy[:], 0.0)
            for i in range(N_WARM):
                pw = psr.tile([128, 512], f32, tag="psr")
                nc.tensor.matmul(pw[:], dummy[:, 0:128], dummy[:], start=True, stop=True)

            # --- constants ---
            ident = cpool.tile([128, 128], f32)
            make_identity(nc, ident[:])
            wt_sb = cpool.tile([128, NCH, 2 * L], f16)
            nc.sync.dma_start(
                out=wt_sb[:], in_=wt_d.ap().rearrange("p (c l) -> p c l", l=2 * L)
            )
            b7_sb = cpool.tile([128, D], bf16)
            nc.sync.dma_start(out=b7_sb[:], in_=b7_d[:, :])
            c_sb = cpool.tile([128, L - 1], f32)
            for l in range(L - 1):
                nc.gpsimd.memset(c_sb[:, l : l + 1], cvals[l])

            for t in range(NT):
                if t == 0:
                    xf = xf0
                elif t == 1:
                    xf = xf1
                elif t == 2:
                    xf = xf2
                else:
                    xf = xfpool.tile([128, TPR * D], f32, tag="xf")
                    nc.sync.dma_start(out=xf[:], in_=x_tile[t])

                # pack f32 (r, d) -> fp16 interleaved (d, r): u32 = row pair
                pk = pkpool.tile([128, TPR * D], f16, tag="pk")
                nc.scalar.copy(
                    out=pk[:].rearrange("p (d r) -> p r d", r=TPR),
                    in_=xf[:].rearrange("p (r d) -> p r d", r=TPR),
                )

                # transpose packed pairs: 8 chunks of u32 [128, 128] via PE
                pk32 = pk[:].bitcast(f32)            # [128, 1024] (bit view)
                psT = pst.tile([128, NCH, 128], f32, tag="pst")
                for c in range(NCH):
                    nc.tensor.transpose(
                        psT[:, c, :], pk32[:, 128 * c : 128 * (c + 1)], ident[:]
                    )

                # PSUM -> SBUF (bit-exact u32 copy)
                xT = xtpool.tile([128, NCH, 128], f32, tag="xt")
                nc.vector.tensor_copy(
                    xT[:].bitcast(u32), psT[:].bitcast(u32)
                )

                # [Uh; Ul*2^11]^T [2L, 256] = sum_c [Wh|Wl]_c.T @ xT_c (fp16)
                ps_u = psu.tile([2 * L, TPR * 128], f32, tag="psu")
                for c in range(NCH):
                    nc.tensor.matmul(
                        ps_u[:], wt_sb[:, c, :], xT[:, c, :].bitcast(f16),
                        start=(c == 0), stop=(c == NCH - 1),
                    )
                ut = spool.tile([2 * L, TPR * 128], f32, tag="ut")
                nc.scalar.copy(ut[:], ps_u[:])

                # U back to row-partition orientation, split even/odd rows
                ut_v = ut[:].rearrange("l (b r) -> l r b", r=TPR)
                yt = ytpool.tile([128, TPR * D], bf16, tag="yt")
                for r in range(TPR):
                    pr = psr.tile([128, 2 * L], f32, tag="psr")
                    nc.tensor.transpose(
                        pr[:], ut_v[:, r, :], ident[0 : 2 * L, 0 : 2 * L]
                    )
                    # u = Uh + 2^-11 * (Ul*2^11)   (free-dim halves of pr)
                    prh = spool.tile([128, L], f32, tag="prh")
                    nc.scalar.copy(prh[:], pr[:, 0:L])
                    uf = spool.tile([128, L], f32, tag="uf")
                    nc.vector.scalar_tensor_tensor(
                        uf[:], pr[:, L : 2 * L], float(2.0 ** -11), prh[:],
                        mybir.AluOpType.mult, mybir.AluOpType.add,
                    )
                    rho0 = spool.tile([128, 1], f32, tag="rho0")
                    nc.vector.tensor_scalar_add(rho0[:], uf[:, 0:1], 1.0)
                    scano = spool.tile([128, L - 1], f32, tag="scan")
                    nc.vector.tensor_tensor_scan(
                        scano[:], uf[:, 1:L], c_sb[:], rho0[:, 0:1],
                        mybir.AluOpType.mult, mybir.AluOpType.add,
                    )
                    # out = x * rho + b7  (bf16 out)
                    nc.vector.scalar_tensor_tensor(
                        yt[:, D * r : D * (r + 1)], xf[:, D * r : D * (r + 1)],
                        scano[:, L - 2 : L - 1], b7_sb[:],
                        mybir.AluOpType.mult, mybir.AluOpType.add,
                    )
                nc.scalar.dma_start(out=y_tile[t], in_=yt[:])

    nc.compile()
    return nc


def kernel(x, W, b):
    global LAST_RESULTS
    x = np.ascontiguousarray(np.asarray(x), dtype=np.float32)
    W = np.ascontiguousarray(np.asarray(W), dtype=np.float32)
    b = np.ascontiguousarray(np.asarray(b), dtype=np.float32)
    assert x.shape == (B, D) and W.shape == (L, D) and b.shape == (L, D)

    import ml_dtypes

    cvals = [float(np.dot(b[l - 1].astype(np.float64), W[l].astype(np.float64)) + 1.0)
             for l in range(1, L)]
    # fp16 hi/lo split of W.T, chunked: per chunk columns [Wh(8) | Wl*2^11(8)]
    wh = W.astype(np.float16)
    wl = ((W.astype(np.float64) - wh.astype(np.float64)) * 2048.0).astype(np.float16)
    wcat = np.concatenate([wh.T.reshape(NCH, 128, L), wl.T.reshape(NCH, 128, L)],
                          axis=2)                       # [NCH, 128, 2L]
    wt = np.ascontiguousarray(wcat.transpose(1, 0, 2).reshape(128, NCH * 2 * L))
    b7r = np.ascontiguousarray(np.broadcast_to(b[L - 1], (128, D))).astype(
        ml_dtypes.bfloat16
    )

    nc = _build(cvals)

    shards = [x[i * RPC : (i + 1) * RPC] for i in range(N_CORES)]
    in_maps = [{"x": s, "wt": wt, "b7r": b7r} for s in shards]
    res = run_bass_kernel_spmd(nc, in_maps, core_ids=list(range(N_CORES)))
    LAST_RESULTS = res
    out = np.concatenate([res.results[i]["y"] for i in range(N_CORES)], axis=0)
    return out.astype(np.float32)
